# revision 1
# baseline (speedup 1.0000x reference)
"""GAT 2-layer GNN kernel for Trainium2, 8 NeuronCores.

Strategy (edge-sharded by destination, "diagonal" slot layout):
  - Nodes are sorted by in-degree (incl. self-loop) and dealt round-robin to
    the 8 cores, so every core owns 6250 destination nodes with a near
    identical degree distribution.  Within a core, destinations are grouped
    into 49 blocks of 128 (SBUF partition = destination).  Edges of a
    destination occupy "g-slots" [dst, g]; blocks are degree-sorted so the
    per-block max degree ~= mean degree (little slot padding).
  - Phase A: each core computes h1 = x@W1 for its own nodes.  The extended
    weight matrix W1ext = [W1 | W1@Asrc | W1@Adst] makes the PE matmul also
    produce the per-node attention terms a_src / a_dst.  Rows [h1|a_src] go
    to a DRAM slice; an AllGather builds the full node table [N, 264].
  - Phase B: per 128-edge-slot chunk, one indirect DMA gathers the source
    rows; DVE computes w = exp(leakyrelu(a_src+a_dst)) * mask and reduces
    w * h over the g axis straight into per-block accumulators; evacuation
    normalizes by sum(w), adds bias, applies ELU, and PE-transposes x2 tiles
    for the next layer's matmul.
  - Phase C/D: same scheme for layer 2 (single head, 32 channels) with the
    table [h2 | a_src2] of 33 floats per node.
  - Softmax is computed without the segment-max subtraction: logits are
    bounded (|l| < ~10) so exp() cannot overflow and the result is
    mathematically identical.
  - All index bookkeeping (permutations, slot->row maps, masks) is done on
    the host; outputs are de-permuted on the host.
"""

import math
import numpy as np

# ---------------------------------------------------------------- problem cfg
N = 50000
E = 800000
F_IN = 128
H = 8
CH = 32
F2 = H * CH  # 256
COUT = 32
NCORES = 8
P = 128
NPC = N // NCORES  # 6250 nodes per core
NBLK = (NPC + P - 1) // P  # 49 destination blocks per core
NPAD = NBLK * P  # 6272 padded rows per core
GCH = 8  # g-slots per chunk
MERGE = 4  # chunks fetched per indirect DMA
MG = GCH * MERGE
ROW1 = F2 + H  # 264: [h1 (256) | a_src1 (8)]
ROW2 = COUT + 1  # 33:  [h2 (32)  | a_src2 (1)]
NEG_SLOPE = 0.2

_CACHE = {}


# ================================================================ host prep
def _prep(edge_index):
    src = np.concatenate(
        [edge_index[0].astype(np.int64), np.arange(N, dtype=np.int64)]
    )
    dst = np.concatenate(
        [edge_index[1].astype(np.int64), np.arange(N, dtype=np.int64)]
    )
    indeg = np.bincount(dst, minlength=N)
    order = np.argsort(-indeg, kind="stable")  # nodes, degree descending
    posn = np.empty(N, np.int64)
    posn[order] = np.arange(N)
    core_of = posn % NCORES
    pos_of = posn // NCORES
    row_of = core_of * NPC + pos_of  # node -> global table row

    # edges sorted by destination row
    er = row_of[dst]
    eorder = np.argsort(er, kind="stable")
    er_sorted = er[eorder]
    src_row_sorted = row_of[src[eorder]].astype(np.int64)
    starts = np.searchsorted(er_sorted, np.arange(N))
    deg_row = np.zeros(N, np.int64)
    deg_row[row_of] = indeg  # degree by row id

    # common chunk schedule (identical across cores by construction)
    Gb = indeg[order[np.arange(NBLK) * P * NCORES]]  # max degree per block
    nch = np.maximum(1, -(-Gb // GCH)).astype(np.int64)
    chunk_bk = [(b, k) for b in range(NBLK) for k in range(int(nch[b]))]
    TC0 = len(chunk_bk)
    NG = -(-TC0 // MERGE)
    TC = NG * MERGE
    chunk_bk += [
        (NBLK - 1, int(nch[NBLK - 1]) + 1 + j) for j in range(TC - TC0)
    ]
    b_arr = np.array([b for b, _ in chunk_bk], np.int64)
    k_arr = np.array([k for _, k in chunk_bk], np.int64)

    d_ar = np.arange(P, dtype=np.int64)
    g_ar = np.arange(GCH, dtype=np.int64)
    locpos = b_arr[:, None, None] * P + d_ar[None, :, None]  # [TC,128,1]
    gs = k_arr[:, None, None] * GCH + g_ar[None, None, :]  # [TC,1,GCH]
    valid_pos = locpos < NPC
    ET = src.shape[0]

    idx_cores, mask_cores, nodes_cores = [], [], []
    for c in range(NCORES):
        glob_rows = np.minimum(c * NPC + locpos, N - 1)
        deg = deg_row[glob_rows]  # [TC,128,1]
        valid = valid_pos & (gs < deg)
        eidx = np.minimum(starts[glob_rows] + gs, ET - 1)
        idxv = np.where(valid, src_row_sorted[eidx], 0).astype(np.int32)
        maskv = valid.astype(np.float32)
        # merged layout [NG, 128, MERGE*GCH]
        idxv = (
            idxv.reshape(NG, MERGE, P, GCH)
            .transpose(0, 2, 1, 3)
            .reshape(NG, P, MG)
        )
        maskv = (
            maskv.reshape(NG, MERGE, P, GCH)
            .transpose(0, 2, 1, 3)
            .reshape(NG, P, MG)
        )
        idx_cores.append(np.ascontiguousarray(idxv))
        mask_cores.append(np.ascontiguousarray(maskv))
        nodes_cores.append(order[np.arange(NPC) * NCORES + c])

    return dict(
        chunk_bk=chunk_bk,
        NG=NG,
        TC=TC,
        idx=idx_cores,
        mask=mask_cores,
        nodes=nodes_cores,
    )


# ================================================================ device prog
def _build_program(NG, chunk_bk):
    from concourse import bass, bacc, mybir, tile

    f32 = mybir.dt.float32
    i32 = mybir.dt.int32
    AF = mybir.ActivationFunctionType
    OP = mybir.AluOpType
    AX = mybir.AxisListType

    nc = bacc.Bacc(
        "TRN2", target_bir_lowering=False, debug=False, num_devices=NCORES
    )

    # -------- I/O
    xT_d = nc.dram_tensor("xT", [F_IN, NPAD], f32, kind="ExternalInput")
    idx_d = nc.dram_tensor("idx", [NG, P, MG], i32, kind="ExternalInput")
    mask_d = nc.dram_tensor("mask", [NG, P, MG], f32, kind="ExternalInput")
    w1e_d = nc.dram_tensor("w1ext", [F_IN, F2 + 2 * H], f32, kind="ExternalInput")
    w2e_d = nc.dram_tensor("w2ext", [F2, COUT + 2], f32, kind="ExternalInput")
    b1_d = nc.dram_tensor("b1rep", [P, F2], f32, kind="ExternalInput")
    b2_d = nc.dram_tensor("b2rep", [P, COUT], f32, kind="ExternalInput")
    id_d = nc.dram_tensor("ident", [P, P], f32, kind="ExternalInput")
    out_d = nc.dram_tensor("out", [NPAD, COUT], f32, kind="ExternalOutput")

    aspace = "Shared" if NCORES > 4 else "Local"
    h1s_d = nc.dram_tensor("h1slice", [NPC, ROW1], f32)
    h1f_d = nc.dram_tensor("h1full", [N, ROW1], f32, addr_space=aspace)
    h2s_d = nc.dram_tensor("h2slice", [NPC, ROW2], f32)
    h2f_d = nc.dram_tensor("h2full", [N, ROW2], f32, addr_space=aspace)

    groups = [[i for i in range(NCORES)]]

    with tile.TileContext(nc) as tc:
        from contextlib import ExitStack

        ctx = ExitStack()
        cpool = ctx.enter_context(tc.tile_pool(name="consts", bufs=1))
        rpool = ctx.enter_context(tc.tile_pool(name="resident", bufs=1))
        apool = ctx.enter_context(tc.tile_pool(name="pha", bufs=3))
        gpool = ctx.enter_context(tc.tile_pool(name="gather", bufs=2))
        wpool = ctx.enter_context(tc.tile_pool(name="work", bufs=3))
        epool = ctx.enter_context(tc.tile_pool(name="evac", bufs=3))
        pspool = ctx.enter_context(tc.tile_pool(name="psum", bufs=2, space="PSUM"))

        # constants
        w1e_t = cpool.tile([F_IN, F2 + 2 * H], f32)
        nc.sync.dma_start(out=w1e_t[:], in_=w1e_d[:])
        w2a_t = cpool.tile([P, COUT + 2], f32)
        nc.sync.dma_start(out=w2a_t[:], in_=w2e_d[0:P, :])
        w2b_t = cpool.tile([P, COUT + 2], f32)
        nc.sync.dma_start(out=w2b_t[:], in_=w2e_d[P : 2 * P, :])
        b1_t = cpool.tile([P, F2], f32)
        nc.sync.dma_start(out=b1_t[:], in_=b1_d[:])
        b2_t = cpool.tile([P, COUT], f32)
        nc.sync.dma_start(out=b2_t[:], in_=b2_d[:])
        id_t = cpool.tile([P, P], f32)
        nc.sync.dma_start(out=id_t[:], in_=id_d[:])

        # resident state
        adst1_all = rpool.tile([P, NBLK * H], f32)
        adst2_all = rpool.tile([P, NBLK], f32)
        x2T0 = rpool.tile([P, NPAD], f32)
        x2T1 = rpool.tile([P, NPAD], f32)

        # ---------------- phase A: h1ext slice
        for b in range(NBLK):
            rv = min(P, NPC - b * P)
            xt = apool.tile([F_IN, P], f32)
            nc.sync.dma_start(out=xt[:], in_=xT_d[:, b * P : (b + 1) * P])
            ps = pspool.tile([P, F2 + 2 * H], f32, tag="psA")
            nc.tensor.matmul(ps[:], lhsT=xt[:], rhs=w1e_t[:], start=True, stop=True)
            h1t = apool.tile([P, ROW1], f32)
            nc.scalar.copy(out=h1t[:], in_=ps[:, 0:ROW1])
            nc.scalar.copy(
                out=adst1_all[:, b * H : (b + 1) * H],
                in_=ps[:, ROW1 : ROW1 + H],
            )
            nc.sync.dma_start(
                out=h1s_d[b * P : b * P + rv, :], in_=h1t[0:rv, :]
            )

        nc.gpsimd.collective_compute(
            "AllGather",
            OP.bypass,
            replica_groups=groups,
            ins=[h1s_d[:]],
            outs=[h1f_d[:]],
        )

        # ---------------- phase B: layer-1 edge aggregation
        acc = {}
        sacc = {}
        nch_of = {}
        for b, k in chunk_bk:
            nch_of[b] = max(nch_of.get(b, 0), k + 1)

        def evac1(b):
            sa, ac = sacc[b], acc[b]
            nc.vector.tensor_scalar_add(sa[:], sa[:], 1e-16)
            rs = epool.tile([P, H], f32, tag="rs1")
            nc.vector.reciprocal(rs[:], sa[:])
            x1 = epool.tile([P, F2], f32, tag="x1")
            nc.vector.tensor_tensor(
                out=x1[:].rearrange("p (h c) -> p h c", c=CH),
                in0=ac[:].rearrange("p (h c) -> p h c", c=CH),
                in1=rs[:].unsqueeze(2).broadcast_to([P, H, CH]),
                op=OP.mult,
            )
            nc.vector.tensor_tensor(out=x1[:], in0=x1[:], in1=b1_t[:], op=OP.add)
            # ELU = exp(min(x,0)) - 1 + max(x,0)
            tmin = epool.tile([P, F2], f32, tag="tmin")
            nc.vector.tensor_scalar_min(tmin[:], x1[:], 0.0)
            texp = epool.tile([P, F2], f32, tag="texp")
            nc.scalar.activation(out=texp[:], in_=tmin[:], func=AF.Exp)
            tmax = epool.tile([P, F2], f32, tag="tmax")
            nc.vector.tensor_scalar(tmax[:], x1[:], 0.0, -1.0, OP.max, OP.add)
            x2 = epool.tile([P, F2], f32, tag="x2")
            nc.vector.tensor_tensor(out=x2[:], in0=texp[:], in1=tmax[:], op=OP.add)
            for half, x2T in ((0, x2T0), (1, x2T1)):
                pst = pspool.tile([P, P], f32, tag="psT")
                nc.tensor.transpose(
                    pst[:], x2[:, half * P : (half + 1) * P], id_t[:]
                )
                nc.scalar.copy(out=x2T[:, b * P : (b + 1) * P], in_=pst[:])

        for gi in range(NG):
            ixt = wpool.tile([P, MG], i32, tag="ixt")
            nc.sync.dma_start(out=ixt[:], in_=idx_d[gi])
            mkt = wpool.tile([P, MG], f32, tag="mkt")
            nc.sync.dma_start(out=mkt[:], in_=mask_d[gi])
            gt = gpool.tile([P, MG, ROW1], f32, tag="g1")
            for m in range(MG):
                nc.gpsimd.indirect_dma_start(
                    out=gt[:, m, :],
                    out_offset=None,
                    in_=h1f_d[:],
                    in_offset=bass.IndirectOffsetOnAxis(ap=ixt[:, m : m + 1], axis=0),
                )
            for j in range(MERGE):
                t = gi * MERGE + j
                b, k = chunk_bk[t]
                sl = slice(j * GCH, (j + 1) * GCH)
                lg = wpool.tile([P, GCH, H], f32, tag="lg")
                nc.vector.tensor_tensor(
                    out=lg[:],
                    in0=gt[:, sl, F2 : F2 + H],
                    in1=adst1_all[:, b * H : (b + 1) * H]
                    .unsqueeze(1)
                    .broadcast_to([P, GCH, H]),
                    op=OP.add,
                )
                lg2 = wpool.tile([P, GCH, H], f32, tag="lg2")
                nc.vector.tensor_scalar_mul(lg2[:], lg[:], NEG_SLOPE)
                nc.vector.tensor_tensor(
                    out=lg[:], in0=lg[:], in1=lg2[:], op=OP.max
                )
                wt = wpool.tile([P, GCH, H], f32, tag="wt")
                nc.scalar.activation(out=wt[:], in_=lg[:], func=AF.Exp)
                nc.vector.tensor_tensor(
                    out=wt[:],
                    in0=wt[:],
                    in1=mkt[:, sl].unsqueeze(2).broadcast_to([P, GCH, H]),
                    op=OP.mult,
                )
                if k == 0:
                    sacc[b] = epool.tile([P, H], f32, tag="sacc", name=f"sacc{b}")
                    acc[b] = epool.tile([P, F2], f32, tag="acc", name=f"acc{b}")
                    st_out, ac_out = sacc[b], acc[b]
                else:
                    st_out = wpool.tile([P, H], f32, tag="st")
                    ac_out = wpool.tile([P, F2], f32, tag="ac")
                nc.vector.tensor_reduce(
                    out=st_out[:],
                    in_=wt[:].transpose([0, 2, 1]),
                    axis=AX.X,
                    op=OP.add,
                )
                msg = wpool.tile([P, GCH, F2], f32, tag="msg")
                nc.vector.tensor_tensor(
                    out=msg[:].rearrange("p g (h c) -> p g h c", c=CH),
                    in0=gt[:, sl, 0:F2].rearrange("p g (h c) -> p g h c", c=CH),
                    in1=wt[:].unsqueeze(3).broadcast_to([P, GCH, H, CH]),
                    op=OP.mult,
                )
                nc.vector.tensor_reduce(
                    out=ac_out[:],
                    in_=msg[:].transpose([0, 2, 1]),
                    axis=AX.X,
                    op=OP.add,
                )
                if k > 0:
                    nc.vector.tensor_tensor(
                        out=sacc[b][:], in0=sacc[b][:], in1=st_out[:], op=OP.add
                    )
                    nc.vector.tensor_tensor(
                        out=acc[b][:], in0=acc[b][:], in1=ac_out[:], op=OP.add
                    )
                if k == nch_of[b] - 1:
                    evac1(b)

        # ---------------- phase C: h2ext slice
        for b in range(NBLK):
            rv = min(P, NPC - b * P)
            ps2 = pspool.tile([P, COUT + 2], f32, tag="psC")
            nc.tensor.matmul(
                ps2[:],
                lhsT=x2T0[:, b * P : (b + 1) * P],
                rhs=w2a_t[:],
                start=True,
                stop=False,
            )
            nc.tensor.matmul(
                ps2[:],
                lhsT=x2T1[:, b * P : (b + 1) * P],
                rhs=w2b_t[:],
                start=False,
                stop=True,
            )
            h2t = apool.tile([P, ROW2], f32, tag="h2t")
            nc.scalar.copy(out=h2t[:], in_=ps2[:, 0:ROW2])
            nc.scalar.copy(
                out=adst2_all[:, b : b + 1], in_=ps2[:, ROW2 : ROW2 + 1]
            )
            nc.sync.dma_start(
                out=h2s_d[b * P : b * P + rv, :], in_=h2t[0:rv, :]
            )

        nc.gpsimd.collective_compute(
            "AllGather",
            OP.bypass,
            replica_groups=groups,
            ins=[h2s_d[:]],
            outs=[h2f_d[:]],
        )

        # ---------------- phase D: layer-2 edge aggregation
        acc2 = {}
        sacc2 = {}

        def evac2(b):
            sa, ac = sacc2[b], acc2[b]
            nc.vector.tensor_scalar_add(sa[:], sa[:], 1e-16)
            rs = epool.tile([P, 1], f32, tag="rs2")
            nc.vector.reciprocal(rs[:], sa[:])
            o1 = epool.tile([P, COUT], f32, tag="o1")
            nc.vector.tensor_tensor(
                out=o1[:],
                in0=ac[:],
                in1=rs[:].broadcast_to([P, COUT]),
                op=OP.mult,
            )
            nc.vector.tensor_tensor(out=o1[:], in0=o1[:], in1=b2_t[:], op=OP.add)
            tmin = epool.tile([P, COUT], f32, tag="tmin2")
            nc.vector.tensor_scalar_min(tmin[:], o1[:], 0.0)
            texp = epool.tile([P, COUT], f32, tag="texp2")
            nc.scalar.activation(out=texp[:], in_=tmin[:], func=AF.Exp)
            tmax = epool.tile([P, COUT], f32, tag="tmax2")
            nc.vector.tensor_scalar(tmax[:], o1[:], 0.0, -1.0, OP.max, OP.add)
            o2 = epool.tile([P, COUT], f32, tag="o2")
            nc.vector.tensor_tensor(out=o2[:], in0=texp[:], in1=tmax[:], op=OP.add)
            nc.sync.dma_start(out=out_d[b * P : (b + 1) * P, :], in_=o2[:])

        for gi in range(NG):
            ixt = wpool.tile([P, MG], i32, tag="ixt2")
            nc.sync.dma_start(out=ixt[:], in_=idx_d[gi])
            mkt = wpool.tile([P, MG], f32, tag="mkt2")
            nc.sync.dma_start(out=mkt[:], in_=mask_d[gi])
            gt = gpool.tile([P, MG, ROW2], f32, tag="g2")
            for m in range(MG):
                nc.gpsimd.indirect_dma_start(
                    out=gt[:, m, :],
                    out_offset=None,
                    in_=h2f_d[:],
                    in_offset=bass.IndirectOffsetOnAxis(ap=ixt[:, m : m + 1], axis=0),
                )
            for j in range(MERGE):
                t = gi * MERGE + j
                b, k = chunk_bk[t]
                sl = slice(j * GCH, (j + 1) * GCH)
                lg = wpool.tile([P, GCH, 1], f32, tag="lgB")
                nc.vector.tensor_tensor(
                    out=lg[:],
                    in0=gt[:, sl, COUT : COUT + 1],
                    in1=adst2_all[:, b : b + 1]
                    .unsqueeze(1)
                    .broadcast_to([P, GCH, 1]),
                    op=OP.add,
                )
                lg2 = wpool.tile([P, GCH, 1], f32, tag="lg2B")
                nc.vector.tensor_scalar_mul(lg2[:], lg[:], NEG_SLOPE)
                nc.vector.tensor_tensor(
                    out=lg[:], in0=lg[:], in1=lg2[:], op=OP.max
                )
                wt = wpool.tile([P, GCH, 1], f32, tag="wtB")
                nc.scalar.activation(out=wt[:], in_=lg[:], func=AF.Exp)
                nc.vector.tensor_tensor(
                    out=wt[:],
                    in0=wt[:],
                    in1=mkt[:, sl].unsqueeze(2),
                    op=OP.mult,
                )
                if k == 0:
                    sacc2[b] = epool.tile([P, 1], f32, tag="sacc2", name=f"sacc2_{b}")
                    acc2[b] = epool.tile([P, COUT], f32, tag="acc2", name=f"acc2_{b}")
                    st_out, ac_out = sacc2[b], acc2[b]
                else:
                    st_out = wpool.tile([P, 1], f32, tag="stB")
                    ac_out = wpool.tile([P, COUT], f32, tag="acB")
                nc.vector.tensor_reduce(
                    out=st_out[:],
                    in_=wt[:].transpose([0, 2, 1]),
                    axis=AX.X,
                    op=OP.add,
                )
                msg = wpool.tile([P, GCH, COUT], f32, tag="msgB")
                nc.vector.tensor_tensor(
                    out=msg[:],
                    in0=gt[:, sl, 0:COUT],
                    in1=wt[:].broadcast_to([P, GCH, COUT]),
                    op=OP.mult,
                )
                nc.vector.tensor_reduce(
                    out=ac_out[:],
                    in_=msg[:].transpose([0, 2, 1]),
                    axis=AX.X,
                    op=OP.add,
                )
                if k > 0:
                    nc.vector.tensor_tensor(
                        out=sacc2[b][:], in0=sacc2[b][:], in1=st_out[:], op=OP.add
                    )
                    nc.vector.tensor_tensor(
                        out=acc2[b][:], in0=acc2[b][:], in1=ac_out[:], op=OP.add
                    )
                if k == nch_of[b] - 1:
                    evac2(b)

        ctx.close()

    nc.compile()
    return nc


# ================================================================ entry point
def kernel(x, edge_index, W1, att_src1, att_dst1, b1, W2, att_src2, att_dst2, b2):
    global LAST_EXEC_TIME_NS
    x = np.asarray(x, np.float32)
    edge_index = np.asarray(edge_index)
    W1 = np.asarray(W1, np.float32)
    W2 = np.asarray(W2, np.float32)

    pr = _prep(edge_index)

    key = (pr["NG"], tuple(pr["chunk_bk"]))
    if key not in _CACHE:
        _CACHE.clear()
        _CACHE[key] = _build_program(pr["NG"], pr["chunk_bk"])
    nc = _CACHE[key]

    # extended weights: [W | W @ Asrc | W @ Adst]
    A1s = np.zeros((F2, H), np.float32)
    A1d = np.zeros((F2, H), np.float32)
    for h in range(H):
        A1s[h * CH : (h + 1) * CH, h] = np.asarray(att_src1, np.float32)[h]
        A1d[h * CH : (h + 1) * CH, h] = np.asarray(att_dst1, np.float32)[h]
    w1ext = np.concatenate([W1, W1 @ A1s, W1 @ A1d], axis=1)
    w2ext = np.concatenate(
        [
            W2,
            W2 @ np.asarray(att_src2, np.float32).reshape(COUT, 1),
            W2 @ np.asarray(att_dst2, np.float32).reshape(COUT, 1),
        ],
        axis=1,
    )
    b1rep = np.broadcast_to(np.asarray(b1, np.float32), (P, F2)).copy()
    b2rep = np.broadcast_to(np.asarray(b2, np.float32), (P, COUT)).copy()
    ident = np.eye(P, dtype=np.float32)

    in_maps = []
    for c in range(NCORES):
        xs = np.zeros((NPAD, F_IN), np.float32)
        xs[:NPC] = x[pr["nodes"][c]]
        in_maps.append(
            dict(
                xT=np.ascontiguousarray(xs.T),
                idx=pr["idx"][c],
                mask=pr["mask"][c],
                w1ext=w1ext,
                w2ext=w2ext,
                b1rep=b1rep,
                b2rep=b2rep,
                ident=ident,
            )
        )

    from concourse.bass_utils import run_bass_kernel_spmd

    res = run_bass_kernel_spmd(
        nc, in_maps, core_ids=list(range(NCORES)), trace=False
    )
    LAST_EXEC_TIME_NS = res.exec_time_ns

    out = np.empty((N, COUT), np.float32)
    for c in range(NCORES):
        out[pr["nodes"][c]] = res.results[c]["out"][:NPC]
    return out


LAST_EXEC_TIME_NS = None

